# revision 12
# baseline (speedup 1.0000x reference)
"""Otsu-threshold binarize (nn_BinarizeLayer) on 8 Trainium2 NeuronCores, v6.

Pipeline (2 SPMD launches, data-parallel over batch):
  L1 stats : reads x (f32, 16 MiB/core) once.  DVE computes exact f32
             min/max via tensor_scalar accum reduces (accum op follows op1);
             scalar writes a bf16 copy of x (xb, 8 MiB/core) + a stride-64
             subsample.  DVE-/DMA-co-bound at ~70-78us.
  host     : combine min/max, coarse histogram of the subsample -> j_hat.
  L2 fused : reads xb (8 MiB/core) and produces TWO u8 planes:
               y    = (xb > T_spec)                       (DVE is_gt)
               code = sat_u8(max(rne(xb*s + B1) - 2^23, 0))   (bin index;
                      u8 conversion saturates at 255 = last-bin-closed)
             scalar: w = xb*s + B1 (f32, rne at 2^23).  DMA-bound ~47us.
  host     : np.bincount(code) = full 256-bin histogram, f64 Otsu argmax
             over ALL 255 splits verifies j_hat; on a miss L2 is relaunched
             with the corrected threshold (same NEFF).

y compares bf16(x) instead of x: ~1.5K boundary pixels flip vs the
reference (rel err ~9e-3, tolerance 2e-2).  The code histogram is
boundary-exact to the same level as the reference's own f32 binning; the
Otsu argmax is robust to the ~100K boundary-fuzz elements (validated
offline: argmax identical, V top-2 gap 4e-6 >> perturbation).
"""

import numpy as np
import ml_dtypes

import concourse.bass as bass
import concourse.mybir as mybir
from concourse.bass_utils import run_bass_kernel_spmd

F32 = mybir.dt.float32
BF16 = mybir.dt.bfloat16
U8 = mybir.dt.uint8
ALU = mybir.AluOpType
ACT = mybir.ActivationFunctionType

NCORES = 8
P = 128
FREE = 32768
SHAPE = (16, 1024, 2048, 1)
NTOT = SHAPE[0] * SHAPE[1] * SHAPE[2] * SHAPE[3]

CS1 = [1024, 3072] + [4096] * 7          # L1 chunks (small ramp-up first)
NC1 = len(CS1)
OFF1 = [0]
for _c in CS1:
    OFF1.append(OFF1[-1] + _c)
assert OFF1[-1] == FREE
SUB64 = FREE // 64          # 512 stride-64 subsample elems / partition

C2 = 4096
NC2 = FREE // C2            # 8 chunks

TWO23 = 8388608.0
BIG = 3.0e38

TRACE = False
EXEC_TIMES_NS = []

_NC_CACHE = {}


def _run(nc, in_maps):
    res = run_bass_kernel_spmd(
        nc, in_maps, core_ids=list(range(NCORES)), trace=TRACE
    )
    if TRACE:
        EXEC_TIMES_NS.append(res.exec_time_ns)
    return res.results


# --------------------------------------------------------------------------
# L1: min/max + bf16 copy + subsample
# --------------------------------------------------------------------------

def _nc_stats():
    if "stats" in _NC_CACHE:
        return _NC_CACHE["stats"]
    nc = bass.Bass()
    x = nc.dram_tensor("x", [P, FREE], F32, kind="ExternalInput")
    mm = nc.dram_tensor("mm", [P, 2 * NC1], F32, kind="ExternalOutput")
    xb = nc.dram_tensor("xb", [P, FREE], BF16, kind="ExternalOutput")
    sub64 = nc.dram_tensor("sub64", [P, SUB64], BF16, kind="ExternalOutput")
    with (
        nc.sbuf_tensor([P, 4, 4096], F32) as xt,
        nc.sbuf_tensor([P, 2, 4096], BF16) as xbt,
        nc.sbuf_tensor([P, 4096], F32) as dmp,
        nc.sbuf_tensor([P, 2 * NC1], F32) as mms,
        nc.sbuf_tensor([P, SUB64], BF16) as s64t,
        nc.semaphore("dma_sem") as dma_sem,
        nc.semaphore("v_sem") as v_sem,
        nc.semaphore("b_sem") as b_sem,
        nc.semaphore("s_sem") as s_sem,
        nc.semaphore("o_sem") as o_sem,
        nc.Block() as block,
    ):
        @block.sync
        def _(sync):
            def store_xb(k):
                sync.wait_ge(b_sem, k + 1)
                sync.dma_start(
                    out=xb[:, OFF1[k]:OFF1[k + 1]],
                    in_=xbt[:, k % 2, :CS1[k]],
                ).then_inc(o_sem, 16)

            for i in range(NC1):
                if i >= 4:
                    # xt slot reuse: DVE + scalar done with chunk i-4
                    sync.wait_ge(v_sem, 2 * (i - 3))
                    sync.wait_ge(s_sem, i - 3)
                sync.dma_start(
                    out=xt[:, i % 4, :CS1[i]], in_=x[:, OFF1[i]:OFF1[i + 1]]
                ).then_inc(dma_sem, 16)
                if i >= 1:
                    store_xb(i - 1)
            store_xb(NC1 - 1)
            sync.wait_ge(v_sem, 2 * NC1)
            sync.dma_start(out=mm[:, :], in_=mms[:, :]).then_inc(dma_sem, 16)
            sync.wait_ge(s_sem, NC1)
            sync.dma_start(out=sub64[:, :], in_=s64t[:, :]).then_inc(dma_sem, 16)
            sync.wait_ge(dma_sem, 16 * (NC1 + 2))
            sync.wait_ge(o_sem, 16 * NC1)

        @block.vector
        def _(vector):
            for i in range(NC1):
                vector.wait_ge(dma_sem, 16 * (i + 1))
                xi = xt[:, i % 4, :CS1[i]]
                # accum reduce op follows op1: per-partition min/max
                vector.tensor_scalar(
                    out=dmp[:, :CS1[i]], in0=xi, scalar1=0.0, scalar2=BIG,
                    op0=ALU.add, op1=ALU.min,
                    accum_out=mms[:, 2 * i:2 * i + 1],
                ).then_inc(v_sem, 1)
                vector.tensor_scalar(
                    out=dmp[:, :CS1[i]], in0=xi, scalar1=0.0, scalar2=-BIG,
                    op0=ALU.add, op1=ALU.max,
                    accum_out=mms[:, 2 * i + 1:2 * i + 2],
                ).then_inc(v_sem, 1)

        @block.scalar
        def _(scalar):
            for i in range(NC1):
                scalar.wait_ge(dma_sem, 16 * (i + 1))
                if i >= 2:
                    scalar.wait_ge(o_sem, 16 * (i - 1))  # xbt slot free
                xi = xt[:, i % 4, :CS1[i]]
                scalar.activation(
                    out=xbt[:, i % 2, :CS1[i]], in_=xi,
                    func=ACT.Copy, bias=0.0, scale=1.0,
                ).then_inc(b_sem, 1)
                s64src = xi.rearrange("p (a s) -> p a s", s=64)
                o64 = OFF1[i] // 64
                n64 = CS1[i] // 64
                scalar.activation(
                    out=s64t[:, o64:o64 + n64], in_=s64src[:, :, 0],
                    func=ACT.Copy, bias=0.0, scale=1.0,
                ).then_inc(s_sem, 1)
    _NC_CACHE["stats"] = nc
    return nc


# --------------------------------------------------------------------------
# L2: fused binarize + bin-code histogram (reads the bf16 copy)
# --------------------------------------------------------------------------

def _nc_fused():
    if "fused" in _NC_CACHE:
        return _NC_CACHE["fused"]
    nc = bass.Bass()
    xb = nc.dram_tensor("xb", [P, FREE], BF16, kind="ExternalInput")
    par = nc.dram_tensor("par", [P, 3], F32, kind="ExternalInput")
    # par: [s, B1(=2^23-0.5-mn*s), T_spec]
    y = nc.dram_tensor("y", [P, FREE], U8, kind="ExternalOutput")
    code = nc.dram_tensor("code", [P, FREE], U8, kind="ExternalOutput")
    with (
        nc.sbuf_tensor([P, 4, C2], BF16) as xt,
        nc.sbuf_tensor([P, 3, C2], F32) as wt,
        nc.sbuf_tensor([P, 3, C2], U8) as yt,
        nc.sbuf_tensor([P, 3, C2], U8) as ct,
        nc.sbuf_tensor([P, 3], F32) as pt,
        nc.semaphore("dma_sem") as dma_sem,
        nc.semaphore("w_sem") as w_sem,
        nc.semaphore("y_sem") as y_sem,
        nc.semaphore("c_sem") as c_sem,
        nc.semaphore("o_sem") as o_sem,
        nc.Block() as block,
    ):
        @block.sync
        def _(sync):
            def store_pair(k):
                sync.wait_ge(y_sem, k + 1)
                sync.dma_start(
                    out=y[:, k * C2:(k + 1) * C2], in_=yt[:, k % 3, :]
                ).then_inc(o_sem, 16)
                sync.wait_ge(c_sem, k + 1)
                sync.dma_start(
                    out=code[:, k * C2:(k + 1) * C2], in_=ct[:, k % 3, :]
                ).then_inc(o_sem, 16)

            sync.dma_start(out=pt[:, :], in_=par[:, :]).then_inc(dma_sem, 16)
            for i in range(NC2):
                if i >= 4:
                    # xt slot reuse: w(i-4) and y(i-4) consumed xb
                    sync.wait_ge(w_sem, i - 3)
                    sync.wait_ge(y_sem, i - 3)
                sync.dma_start(
                    out=xt[:, i % 4, :], in_=xb[:, i * C2:(i + 1) * C2]
                ).then_inc(dma_sem, 16)
                if i >= 2:
                    store_pair(i - 2)
            for k in range(NC2 - 2, NC2):
                store_pair(k)
            sync.wait_ge(dma_sem, 16 * (NC2 + 1))
            sync.wait_ge(o_sem, 16 * 2 * NC2)

        @block.scalar
        def _(scalar):
            scalar.wait_ge(dma_sem, 16)
            for i in range(NC2):
                scalar.wait_ge(dma_sem, 16 * (i + 2))
                if i >= 3:
                    # wt slot reuse: DVE code(i-3) consumed w
                    scalar.wait_ge(c_sem, i - 2)
                # w = rne(xb*s + B1): integer-valued f32 at 2^23 magnitude
                scalar.activation(
                    out=wt[:, i % 3, :], in_=xt[:, i % 4, :],
                    func=ACT.Identity, bias=pt[:, 1:2], scale=pt[:, 0:1],
                ).then_inc(w_sem, 1)

        @block.vector
        def _(vector):
            vector.wait_ge(dma_sem, 16)
            for i in range(NC2):
                vector.wait_ge(dma_sem, 16 * (i + 2))
                if i >= 3:
                    vector.wait_ge(o_sem, 16 * 2 * (i - 2))  # yt/ct slots
                # y = (xb > T_spec) as u8
                vector.tensor_scalar(
                    out=yt[:, i % 3, :], in0=xt[:, i % 4, :],
                    scalar1=pt[:, 2:3], scalar2=None,
                    op0=ALU.is_gt).then_inc(y_sem, 1)
                # code = sat_u8(max(w - 2^23, 0)): bin index
                vector.wait_ge(w_sem, i + 1)
                vector.tensor_scalar(
                    out=ct[:, i % 3, :], in0=wt[:, i % 3, :],
                    scalar1=TWO23, scalar2=0.0,
                    op0=ALU.subtract, op1=ALU.max).then_inc(c_sem, 1)
    _NC_CACHE["fused"] = nc
    return nc


# --------------------------------------------------------------------------
# host-side otsu math (replicates reference.py numerics)
# --------------------------------------------------------------------------

def _edges_centers(mn, mx):
    """Replicate jnp.histogram's f32 bin edges + reference centers."""
    step = np.arange(256, dtype=np.float32) / np.float32(256.0)
    out = (mn * (np.float32(1.0) - step) + mx * step).astype(np.float32)
    edges = np.concatenate([out, np.asarray([mx], dtype=np.float32)])
    centers = (np.float32(0.5) * (edges[:-1] + edges[1:])).astype(np.float32)
    return edges, centers


def _otsu_argmax(cnt, centers):
    """f64 Otsu argmax from 256-bin counts (reference V formula)."""
    cnt = np.asarray(cnt, dtype=np.float64)
    c64 = centers.astype(np.float64)
    w1 = np.cumsum(cnt)
    w2 = np.cumsum(cnt[::-1])[::-1]
    cs = np.cumsum(cnt * c64)
    csr = np.cumsum((cnt * c64)[::-1])[::-1]
    m1 = cs / np.maximum(w1, 1.0)
    m2 = csr / np.maximum(w2, 1.0)
    v = w1[:-1] * w2[1:] * (m1[:-1] - m2[1:]) ** 2
    return int(np.argmax(v))


# --------------------------------------------------------------------------
# main entry
# --------------------------------------------------------------------------

def kernel(inputs):
    x = np.asarray(inputs)
    assert x.shape == SHAPE, x.shape
    x = np.ascontiguousarray(x, dtype=np.float32)
    xs = x.reshape(NCORES, P, FREE)
    shards = [xs[c] for c in range(NCORES)]

    # ---- L1: min/max + bf16 copy + subsample ----
    r = _run(_nc_stats(), [{"x": s} for s in shards])
    mm = np.stack([r[c]["mm"] for c in range(NCORES)])
    xbs = [r[c]["xb"] for c in range(NCORES)]
    s64 = np.stack([r[c]["sub64"] for c in range(NCORES)])
    mn = np.float32(mm[:, :, 0::2].min())
    mx = np.float32(mm[:, :, 1::2].max())
    if not np.isfinite(mn) or not np.isfinite(mx) or mn == mx:
        return np.zeros(SHAPE, dtype=np.float32)

    scale = np.float32(256.0) / (mx - mn)
    edges, centers = _edges_centers(mn, mx)

    # ---- host: coarse histogram of the subsample -> j_hat ----
    xsub = s64.astype(np.float32).ravel()
    cnt_est, _ = np.histogram(xsub, bins=256, range=(float(mn), float(mx)))
    j_hat = _otsu_argmax(cnt_est, centers)

    # ---- L2: binarize + bin-code histogram (with retry) ----
    b1 = np.float32(TWO23) - np.float32(0.5) - np.float32(mn) * scale

    y = None
    j_spec = j_hat
    for _attempt in range(4):
        par = np.zeros((P, 3), dtype=np.float32)
        par[:, 0] = scale
        par[:, 1] = b1
        par[:, 2] = np.float32(centers[j_spec])
        r = _run(_nc_fused(),
                 [{"xb": xbs[c], "par": par} for c in range(NCORES)])
        codes = np.stack([r[c]["code"] for c in range(NCORES)])
        cnt = np.bincount(codes.ravel(), minlength=256)[:256]
        jbest = _otsu_argmax(cnt, centers)
        if jbest == j_spec:
            y = np.stack([r[c]["y"] for c in range(NCORES)])
            break
        j_spec = jbest       # speculation missed; relaunch with exact argmax
    assert y is not None
    return y.astype(np.float32).reshape(SHAPE)


# revision 13
# speedup vs baseline: 1.0661x; 1.0661x over previous
"""Otsu-threshold binarize (nn_BinarizeLayer) on 8 Trainium2 NeuronCores, v6.

Pipeline (2 SPMD launches, data-parallel over batch):
  L1 stats : reads x (f32, 16 MiB/core) once.  DVE computes exact f32
             min/max via tensor_scalar accum reduces (accum op follows op1);
             scalar writes a bf16 copy of x (xb, 8 MiB/core) + a stride-64
             subsample.  DVE-/DMA-co-bound at ~70-78us.
  host     : combine min/max, coarse histogram of the subsample -> j_hat.
  L2 fused : reads xb (8 MiB/core) and produces TWO u8 planes:
               y    = (xb > T_spec)                       (DVE is_gt)
               code = sat_u8(max(rne(xb*s + B1) - 2^23, 0))   (bin index;
                      u8 conversion saturates at 255 = last-bin-closed)
             scalar: w = xb*s + B1 (f32, rne at 2^23).  DMA-bound ~47us.
  host     : np.bincount(code) = full 256-bin histogram, f64 Otsu argmax
             over ALL 255 splits verifies j_hat; on a miss L2 is relaunched
             with the corrected threshold (same NEFF).

y compares bf16(x) instead of x: ~1.5K boundary pixels flip vs the
reference (rel err ~9e-3, tolerance 2e-2).  The code histogram is
boundary-exact to the same level as the reference's own f32 binning; the
Otsu argmax is robust to the ~100K boundary-fuzz elements (validated
offline: argmax identical, V top-2 gap 4e-6 >> perturbation).
"""

import numpy as np
import ml_dtypes

import concourse.bass as bass
import concourse.mybir as mybir
from concourse.bass_utils import run_bass_kernel_spmd

F32 = mybir.dt.float32
BF16 = mybir.dt.bfloat16
U8 = mybir.dt.uint8
ALU = mybir.AluOpType
ACT = mybir.ActivationFunctionType

NCORES = 8
P = 128
FREE = 32768
SHAPE = (16, 1024, 2048, 1)
NTOT = SHAPE[0] * SHAPE[1] * SHAPE[2] * SHAPE[3]

CS1 = [4096] * 8                         # L1 chunks
NC1 = len(CS1)
OFF1 = [0]
for _c in CS1:
    OFF1.append(OFF1[-1] + _c)
assert OFF1[-1] == FREE
SUB64 = FREE // 64          # 512 stride-64 subsample elems / partition

C2 = 4096
NC2 = FREE // C2            # 8 chunks

TWO23 = 8388608.0
BIG = 3.0e38

TRACE = False
EXEC_TIMES_NS = []

_NC_CACHE = {}


def _run(nc, in_maps):
    res = run_bass_kernel_spmd(
        nc, in_maps, core_ids=list(range(NCORES)), trace=TRACE
    )
    if TRACE:
        EXEC_TIMES_NS.append(res.exec_time_ns)
    return res.results


# --------------------------------------------------------------------------
# L1: min/max + bf16 copy + subsample
# --------------------------------------------------------------------------

def _nc_stats():
    if "stats" in _NC_CACHE:
        return _NC_CACHE["stats"]
    nc = bass.Bass()
    x = nc.dram_tensor("x", [P, FREE], F32, kind="ExternalInput")
    mm = nc.dram_tensor("mm", [P, 2 * NC1], F32, kind="ExternalOutput")
    xb = nc.dram_tensor("xb", [P, FREE], BF16, kind="ExternalOutput")
    sub64 = nc.dram_tensor("sub64", [P, SUB64], BF16, kind="ExternalOutput")
    with (
        nc.sbuf_tensor([P, 4, 4096], F32) as xt,
        nc.sbuf_tensor([P, 2, 4096], BF16) as xbt,
        nc.sbuf_tensor([P, 4096], F32) as dmp,
        nc.sbuf_tensor([P, 2 * NC1], F32) as mms,
        nc.sbuf_tensor([P, SUB64], BF16) as s64t,
        nc.semaphore("dma_sem") as dma_sem,
        nc.semaphore("v_sem") as v_sem,
        nc.semaphore("b_sem") as b_sem,
        nc.semaphore("s_sem") as s_sem,
        nc.semaphore("o_sem") as o_sem,
        nc.Block() as block,
    ):
        @block.sync
        def _(sync):
            def store_xb(k):
                sync.wait_ge(b_sem, k + 1)
                sync.dma_start(
                    out=xb[:, OFF1[k]:OFF1[k + 1]],
                    in_=xbt[:, k % 2, :CS1[k]],
                ).then_inc(o_sem, 16)

            for i in range(NC1):
                if i >= 4:
                    # xt slot reuse: DVE + scalar done with chunk i-4
                    sync.wait_ge(v_sem, 2 * (i - 3))
                    sync.wait_ge(s_sem, i - 3)
                sync.dma_start(
                    out=xt[:, i % 4, :CS1[i]], in_=x[:, OFF1[i]:OFF1[i + 1]]
                ).then_inc(dma_sem, 16)
                if i >= 1:
                    store_xb(i - 1)
            store_xb(NC1 - 1)
            sync.wait_ge(v_sem, 2 * NC1)
            sync.dma_start(out=mm[:, :], in_=mms[:, :]).then_inc(dma_sem, 16)
            sync.wait_ge(s_sem, NC1)
            sync.dma_start(out=sub64[:, :], in_=s64t[:, :]).then_inc(dma_sem, 16)
            sync.wait_ge(dma_sem, 16 * (NC1 + 2))
            sync.wait_ge(o_sem, 16 * NC1)

        @block.vector
        def _(vector):
            for i in range(NC1):
                vector.wait_ge(dma_sem, 16 * (i + 1))
                xi = xt[:, i % 4, :CS1[i]]
                # accum reduce op follows op1: per-partition min/max
                vector.tensor_scalar(
                    out=dmp[:, :CS1[i]], in0=xi, scalar1=0.0, scalar2=BIG,
                    op0=ALU.add, op1=ALU.min,
                    accum_out=mms[:, 2 * i:2 * i + 1],
                ).then_inc(v_sem, 1)
                vector.tensor_scalar(
                    out=dmp[:, :CS1[i]], in0=xi, scalar1=0.0, scalar2=-BIG,
                    op0=ALU.add, op1=ALU.max,
                    accum_out=mms[:, 2 * i + 1:2 * i + 2],
                ).then_inc(v_sem, 1)

        @block.scalar
        def _(scalar):
            for i in range(NC1):
                scalar.wait_ge(dma_sem, 16 * (i + 1))
                if i >= 2:
                    scalar.wait_ge(o_sem, 16 * (i - 1))  # xbt slot free
                xi = xt[:, i % 4, :CS1[i]]
                scalar.activation(
                    out=xbt[:, i % 2, :CS1[i]], in_=xi,
                    func=ACT.Copy, bias=0.0, scale=1.0,
                ).then_inc(b_sem, 1)
                s64src = xi.rearrange("p (a s) -> p a s", s=64)
                o64 = OFF1[i] // 64
                n64 = CS1[i] // 64
                scalar.activation(
                    out=s64t[:, o64:o64 + n64], in_=s64src[:, :, 0],
                    func=ACT.Copy, bias=0.0, scale=1.0,
                ).then_inc(s_sem, 1)
    _NC_CACHE["stats"] = nc
    return nc


# --------------------------------------------------------------------------
# L2: fused binarize + bin-code histogram (reads the bf16 copy)
# --------------------------------------------------------------------------

def _nc_fused():
    if "fused" in _NC_CACHE:
        return _NC_CACHE["fused"]
    nc = bass.Bass()
    xb = nc.dram_tensor("xb", [P, FREE], BF16, kind="ExternalInput")
    par = nc.dram_tensor("par", [P, 3], F32, kind="ExternalInput")
    # par: [s, B1(=2^23-0.5-mn*s), T_spec]
    y = nc.dram_tensor("y", [P, FREE], U8, kind="ExternalOutput")
    code = nc.dram_tensor("code", [P, FREE], U8, kind="ExternalOutput")
    with (
        nc.sbuf_tensor([P, 4, C2], BF16) as xt,
        nc.sbuf_tensor([P, 3, C2], F32) as wt,
        nc.sbuf_tensor([P, 3, C2], U8) as yt,
        nc.sbuf_tensor([P, 3, C2], U8) as ct,
        nc.sbuf_tensor([P, 3], F32) as pt,
        nc.semaphore("dma_sem") as dma_sem,
        nc.semaphore("w_sem") as w_sem,
        nc.semaphore("y_sem") as y_sem,
        nc.semaphore("c_sem") as c_sem,
        nc.semaphore("o_sem") as o_sem,
        nc.Block() as block,
    ):
        @block.sync
        def _(sync):
            def store_pair(k):
                sync.wait_ge(y_sem, k + 1)
                sync.dma_start(
                    out=y[:, k * C2:(k + 1) * C2], in_=yt[:, k % 3, :]
                ).then_inc(o_sem, 16)
                sync.wait_ge(c_sem, k + 1)
                sync.dma_start(
                    out=code[:, k * C2:(k + 1) * C2], in_=ct[:, k % 3, :]
                ).then_inc(o_sem, 16)

            sync.dma_start(out=pt[:, :], in_=par[:, :]).then_inc(dma_sem, 16)
            for i in range(NC2):
                if i >= 4:
                    # xt slot reuse: w(i-4) and y(i-4) consumed xb
                    sync.wait_ge(w_sem, i - 3)
                    sync.wait_ge(y_sem, i - 3)
                sync.dma_start(
                    out=xt[:, i % 4, :], in_=xb[:, i * C2:(i + 1) * C2]
                ).then_inc(dma_sem, 16)
                if i >= 2:
                    store_pair(i - 2)
            for k in range(NC2 - 2, NC2):
                store_pair(k)
            sync.wait_ge(dma_sem, 16 * (NC2 + 1))
            sync.wait_ge(o_sem, 16 * 2 * NC2)

        @block.scalar
        def _(scalar):
            scalar.wait_ge(dma_sem, 16)
            for i in range(NC2):
                scalar.wait_ge(dma_sem, 16 * (i + 2))
                if i >= 3:
                    # wt slot reuse: DVE code(i-3) consumed w
                    scalar.wait_ge(c_sem, i - 2)
                # w = rne(xb*s + B1): integer-valued f32 at 2^23 magnitude
                scalar.activation(
                    out=wt[:, i % 3, :], in_=xt[:, i % 4, :],
                    func=ACT.Identity, bias=pt[:, 1:2], scale=pt[:, 0:1],
                ).then_inc(w_sem, 1)

        @block.vector
        def _(vector):
            vector.wait_ge(dma_sem, 16)
            for i in range(NC2):
                vector.wait_ge(dma_sem, 16 * (i + 2))
                if i >= 3:
                    vector.wait_ge(o_sem, 16 * 2 * (i - 2))  # yt/ct slots
                # y = (xb > T_spec) as u8
                vector.tensor_scalar(
                    out=yt[:, i % 3, :], in0=xt[:, i % 4, :],
                    scalar1=pt[:, 2:3], scalar2=None,
                    op0=ALU.is_gt).then_inc(y_sem, 1)
                # code = sat_u8(max(w - 2^23, 0)): bin index
                vector.wait_ge(w_sem, i + 1)
                vector.tensor_scalar(
                    out=ct[:, i % 3, :], in0=wt[:, i % 3, :],
                    scalar1=TWO23, scalar2=0.0,
                    op0=ALU.subtract, op1=ALU.max).then_inc(c_sem, 1)
    _NC_CACHE["fused"] = nc
    return nc


# --------------------------------------------------------------------------
# host-side otsu math (replicates reference.py numerics)
# --------------------------------------------------------------------------

def _edges_centers(mn, mx):
    """Replicate jnp.histogram's f32 bin edges + reference centers."""
    step = np.arange(256, dtype=np.float32) / np.float32(256.0)
    out = (mn * (np.float32(1.0) - step) + mx * step).astype(np.float32)
    edges = np.concatenate([out, np.asarray([mx], dtype=np.float32)])
    centers = (np.float32(0.5) * (edges[:-1] + edges[1:])).astype(np.float32)
    return edges, centers


def _otsu_argmax(cnt, centers):
    """f64 Otsu argmax from 256-bin counts (reference V formula)."""
    cnt = np.asarray(cnt, dtype=np.float64)
    c64 = centers.astype(np.float64)
    w1 = np.cumsum(cnt)
    w2 = np.cumsum(cnt[::-1])[::-1]
    cs = np.cumsum(cnt * c64)
    csr = np.cumsum((cnt * c64)[::-1])[::-1]
    m1 = cs / np.maximum(w1, 1.0)
    m2 = csr / np.maximum(w2, 1.0)
    v = w1[:-1] * w2[1:] * (m1[:-1] - m2[1:]) ** 2
    return int(np.argmax(v))


# --------------------------------------------------------------------------
# main entry
# --------------------------------------------------------------------------

def kernel(inputs):
    x = np.asarray(inputs)
    assert x.shape == SHAPE, x.shape
    x = np.ascontiguousarray(x, dtype=np.float32)
    xs = x.reshape(NCORES, P, FREE)
    shards = [xs[c] for c in range(NCORES)]

    # ---- L1: min/max + bf16 copy + subsample ----
    r = _run(_nc_stats(), [{"x": s} for s in shards])
    mm = np.stack([r[c]["mm"] for c in range(NCORES)])
    xbs = [r[c]["xb"] for c in range(NCORES)]
    s64 = np.stack([r[c]["sub64"] for c in range(NCORES)])
    mn = np.float32(mm[:, :, 0::2].min())
    mx = np.float32(mm[:, :, 1::2].max())
    if not np.isfinite(mn) or not np.isfinite(mx) or mn == mx:
        return np.zeros(SHAPE, dtype=np.float32)

    scale = np.float32(256.0) / (mx - mn)
    edges, centers = _edges_centers(mn, mx)

    # ---- host: coarse histogram of the subsample -> j_hat ----
    xsub = s64.astype(np.float32).ravel()
    cnt_est, _ = np.histogram(xsub, bins=256, range=(float(mn), float(mx)))
    j_hat = _otsu_argmax(cnt_est, centers)

    # ---- L2: binarize + bin-code histogram (with retry) ----
    b1 = np.float32(TWO23) - np.float32(0.5) - np.float32(mn) * scale

    y = None
    j_spec = j_hat
    for _attempt in range(4):
        par = np.zeros((P, 3), dtype=np.float32)
        par[:, 0] = scale
        par[:, 1] = b1
        par[:, 2] = np.float32(centers[j_spec])
        r = _run(_nc_fused(),
                 [{"xb": xbs[c], "par": par} for c in range(NCORES)])
        codes = np.stack([r[c]["code"] for c in range(NCORES)])
        cnt = np.bincount(codes.ravel(), minlength=256)[:256]
        jbest = _otsu_argmax(cnt, centers)
        if jbest == j_spec:
            y = np.stack([r[c]["y"] for c in range(NCORES)])
            break
        j_spec = jbest       # speculation missed; relaunch with exact argmax
    assert y is not None
    return y.astype(np.float32).reshape(SHAPE)
